# revision 52
# baseline (speedup 1.0000x reference)
"""TRN2 Bass kernel for nn_MAD_4612794876395 (retrieval_knn).

Math: with dist = softmax_k(-||pos_d - pos_r||) and sum_k dist = 1, the
reference output collapses to
    out[b,c] = wmem@adapt_w + adapt_b + wdiff@field_b.reshape(H,C)
             + sum_h wdiff[b,h] * (date@field_w)[b, h*C+c]
where wdiff[b,h] = sum_k dist[b,k]*diff[b,k,h].  The last term is 137 GFLOP
and is computed on 8 NeuronCores, tensor-parallel over field_w's 65536
columns (64 h-values per core), as bf16 matmuls (PE) whose PSUM output is
drained to SBUF by the Activation engine and h-contracted against wdiff by
the Vector engine (scalar_tensor_tensor).  Inputs are pre-tiled on the host
into partition-major [128, ...] layouts so each logical load is one DMA.
The small terms are host numpy.
"""
import sys

try:
    import concourse  # noqa: F401
except ImportError:
    sys.path.insert(0, "/opt/trn_rl_repo")

import numpy as np

N_DATA, F, H, C, K, B = 100000, 512, 512, 128, 8, 2048
NCORES = 8
HSH = H // NCORES          # 64 h-values per core
SH = HSH * C               # 8192 field_w cols per core
P = 128
NB = B // P                # 16 b-tiles
NS = SH // 512             # 16 n-slices of 512 cols (4 h each)

_NC = None


def _build():
    import concourse.mybir as mybir
    import concourse.tile as tile
    from concourse import bacc

    nc = bacc.Bacc(None, target_bir_lowering=False, debug=False)
    # all inputs pre-tiled host-side to partition-major layouts
    dateT = nc.dram_tensor("dateT", [P, 4, B], mybir.dt.bfloat16,
                           kind="ExternalInput")
    wdiff = nc.dram_tensor("wdiff", [P, NB, HSH], mybir.dt.float32,
                           kind="ExternalInput")
    fw = nc.dram_tensor("fw", [P, 4, SH], mybir.dt.bfloat16,
                        kind="ExternalInput")
    # boot = [dateT cols 0:128 | fw slice 0] so unit (n=0,t=0) needs one DMA
    boot = nc.dram_tensor("boot", [P, 2560], mybir.dt.bfloat16,
                          kind="ExternalInput")
    partial = nc.dram_tensor("partial", [B, C], mybir.dt.float32,
                             kind="ExternalOutput")

    with tile.TileContext(nc) as tc:
        with (
            tc.tile_pool(name="const", bufs=1) as cp,
            tc.tile_pool(name="fwp", bufs=4) as fwp,
            tc.tile_pool(name="gsp", bufs=8) as gsp,
            tc.tile_pool(name="ps2", bufs=8, space="PSUM") as ps2,
        ):
            # boot tile carries b-tile 0 of dateT plus fw slice 0: the
            # whole first unit becomes runnable off a single DMA.
            bt = cp.tile([P, 2560], mybir.dt.bfloat16, name="boot")
            nc.sync.dma_start(bt[:], boot[:])

            # warm the PE clock while the first DMAs are in flight: dummy
            # matmuls on scratch data keep the PE busy so its clock is
            # fully ramped when real data lands (~4.5us in).
            wl = cp.tile([P, P], mybir.dt.bfloat16, name="warm_l")
            wr = cp.tile([P, 512], mybir.dt.bfloat16, name="warm_r")
            nc.gpsimd.memset(wl[:], 0.0)
            nc.gpsimd.memset(wr[:], 0.0)
            for _ in range(7):
                wp = ps2.tile([P, 512], mybir.dt.float32, name="g", tag="g")
                nc.tensor.matmul(wp[:], wl[:], wr[:], start=True, stop=True)

            # dateT resident in bf16, DMA'd in b-chunks so the first
            # b-tiles' matmuls can start as soon as chunk 0 lands.
            dr = cp.tile([P, 4, B], mybir.dt.bfloat16, name="d_r")
            wd = cp.tile([P, NB, HSH], mybir.dt.float32, name="wd")
            bounds = [128, 512, 1024, 1536, 2048]
            for ch in range(4):
                lo, hi = bounds[ch], bounds[ch + 1]
                nc.sync.dma_start(dr[:, :, lo:hi], dateT[:, :, lo:hi])
                if ch == 1:
                    nc.sync.dma_start(wd[:], wdiff[:])
            # single wide accumulator [128, NB, C]
            A = cp.tile([P, NB, C], mybir.dt.float32, name="acc")
            nc.gpsimd.memset(A[:], 0.0)

            for n in range(NS):
                if n == 0:
                    fwr = None
                else:
                    fwr = fwp.tile([P, 4, 512], mybir.dt.bfloat16, name="f_r",
                                   tag="f_r")
                    nc.sync.dma_start(fwr[:], fw[:, :, n * 512:(n + 1) * 512])
                # the last slice runs half-width units: smaller drains
                # shrink the matmul->drain->contract pipeline offset right
                # where it turns into the end-of-kernel tail
                for t in range(NB):
                    parts = ([(0, 256), (256, 512)]
                             if n == NS - 1 and t >= 8 else [(0, 512)])
                    for (c0, c1) in parts:
                        w = c1 - c0
                        g = ps2.tile([P, 512], mybir.dt.float32, name="g",
                                     tag="g")
                        for fc in range(4):
                            lhs = (bt[:, fc * 640:fc * 640 + 128] if t == 0
                                   else dr[:, fc, t * P:(t + 1) * P])
                            rhs = (bt[:, fc * 640 + 128 + c0:
                                      fc * 640 + 128 + c1] if n == 0
                                   else fwr[:, fc, c0:c1])
                            nc.tensor.matmul(g[:, 0:w], lhs, rhs,
                                             start=(fc == 0), stop=(fc == 3))
                        gs = gsp.tile([P, 512], mybir.dt.float32, name="gs",
                                      tag="gs")
                        nc.scalar.copy(gs[:, 0:w], g[:, 0:w])
                        for l in range(c0 // C, c1 // C):
                            hcol = 4 * n + l
                            nc.vector.scalar_tensor_tensor(
                                out=A[:, t, :],
                                in0=gs[:, (l - c0 // C) * C:
                                        (l - c0 // C + 1) * C],
                                scalar=wd[:, t, hcol:hcol + 1],
                                in1=A[:, t, :],
                                op0=mybir.AluOpType.mult,
                                op1=mybir.AluOpType.add,
                            )
            # drain accumulators: 4 DMAs of 4 b-tiles each so the first can
            # overlap the tail of the h-contraction
            pv = partial[:, :].rearrange("(t p) c -> p t c", p=P)
            for lo, hi in [(0, 4), (4, 8), (8, 12), (12, 15), (15, 16)]:
                nc.sync.dma_start(pv[:, lo:hi, :], A[:, lo:hi, :])
    nc.finalize()
    return nc


def _prep_in_maps(date, field_w, wdiff):
    """Pre-tile inputs to the partition-major device layouts."""
    import ml_dtypes

    bf16 = ml_dtypes.bfloat16
    # dateT[p, fc, b] = date[b, fc*128+p]
    dateTp = np.ascontiguousarray(
        date.T.reshape(4, P, B).transpose(1, 0, 2)).astype(bf16)
    fw16 = field_w.astype(bf16)
    in_maps = []
    for i in range(NCORES):
        # fw[p, fc, col] = field_w[fc*128+p, i*SH+col]
        fwp = np.ascontiguousarray(
            fw16[:, i * SH:(i + 1) * SH].reshape(4, P, SH).transpose(1, 0, 2))
        # wd[p, t, h] = wdiff[t*128+p, i*HSH+h]
        wdp = np.ascontiguousarray(
            wdiff[:, i * HSH:(i + 1) * HSH].reshape(NB, P, HSH)
            .transpose(1, 0, 2))
        bootp = np.ascontiguousarray(
            np.concatenate([dateTp[:, :, 0:128], fwp[:, :, 0:512]],
                           axis=2)).reshape(P, 2560)
        in_maps.append({"dateT": dateTp, "wdiff": wdp, "fw": fwp,
                        "boot": bootp})
    return in_maps


def kernel(idx, date, train_dates, mem, train_nns, pos_w, pos_b, field_w,
           field_b, adapt_w, adapt_b):
    global _NC
    from concourse.bass_utils import run_bass_kernel_spmd

    idx = np.asarray(idx)
    date = np.asarray(date, dtype=np.float32)
    train_dates = np.asarray(train_dates, dtype=np.float32)
    mem = np.asarray(mem, dtype=np.float32)
    train_nns = np.asarray(train_nns)
    pos_w = np.asarray(pos_w, dtype=np.float32)
    pos_b = np.asarray(pos_b, dtype=np.float32)
    field_w = np.asarray(field_w, dtype=np.float32)
    field_b = np.asarray(field_b, dtype=np.float32)
    adapt_w = np.asarray(adapt_w, dtype=np.float32)
    adapt_b = np.asarray(adapt_b, dtype=np.float32)

    # ---- host phase 1 (small): dist, wdiff, const terms ----
    refs = train_nns[idx]                                   # [B, K]
    pos_d = date @ pos_w + pos_b                            # [B, H]
    pos_r = (train_dates[refs.reshape(-1)] @ pos_w + pos_b).reshape(B, K, H)
    diff = pos_d[:, None, :] - pos_r                        # [B, K, H]
    norm = np.sqrt((diff * diff).sum(-1))                   # [B, K]
    m = norm.min(axis=1, keepdims=True)
    e = np.exp(m - norm)
    dist = e / e.sum(axis=1, keepdims=True)                 # [B, K]
    wdiff = np.einsum("bk,bkh->bh", dist, diff).astype(np.float32)
    wmem = np.einsum("bk,bkc->bc", dist, mem[refs]).astype(np.float32)
    const = wmem @ adapt_w + adapt_b + wdiff @ field_b.reshape(H, C)

    # ---- device phase 2: grad-term, TP over the 65536 dim ----
    if _NC is None:
        _NC = _build()
    in_maps = _prep_in_maps(date, field_w, wdiff)
    res = run_bass_kernel_spmd(_NC, in_maps, core_ids=list(range(NCORES)))
    grad_term = np.zeros((B, C), dtype=np.float32)
    for i in range(NCORES):
        grad_term += res.results[i]["partial"]
    return (const + grad_term).astype(np.float32)
